# revision 3
# baseline (speedup 1.0000x reference)
"""Causal single-head attention (B=4, S=2048, D=1024) on 8 trn2 NeuronCores.

Sharding: 8 cores = 4 batches x 2 sequence-shards. Queries are split into
eight 256-row chunks per batch; core (b, p) handles chunks p, p+2, p+4, p+6
of batch b (interleaving balances causal work exactly). K/V projections are
NOT duplicated: core (b, p) projects K/V only for keys [1024p, 1024p+1024),
then the pair exchanges halves with four pipelined 1MB pairwise AllGathers
(DRAM bounce) that complete while the PE continues with V/Q projections.

DMA discipline (the rings, not bytes, were the bottleneck):
  - every 2MB input is pre-laid-out on the host partition-major so it loads
    with ONE dma (16KB contiguous per partition run)
  - every CC input copy / readback is one DMA with 8KB runs (staging tiles
    kstg/vstg; wide ktall/vall layouts)
  - all CC readbacks sit at the END of phase 1 so the in-order sync queue
    never blocks independent work behind a collective wait
  - wq/xtq loads are emitted behind the first CC input copy: they dispatch
    ~25us in, keeping rings clear for the K-half load + exchange
"""

import numpy as np
import ml_dtypes
from contextlib import ExitStack

import concourse.bacc as bacc
import concourse.bass as bass
import concourse.mybir as mybir
import concourse.tile as tile
from concourse import bass_utils

bf16 = ml_dtypes.bfloat16
f32 = np.float32

B, S, D = 4, 2048, 1024
E = D
N_CORES = 8
QCH = 256          # query chunk rows (per-core local chunk)
NCH = 4            # local chunks per core
SQ = QCH * NCH     # 1024 query rows per core
HALF = S // 2      # keys projected locally per core
DT = D // 128      # 8 d-tiles
ET = E // 128      # 8 e-tiles
KT = S // 128      # 16 key tiles (global)
CC_GROUPS = [[0, 1], [2, 3], [4, 5], [6, 7]]

_CACHE = {}


def _kcol(et, kt):
    """Column of global K tile (et, kt) in the ktall layout [c][r][et][512]."""
    r, k8 = divmod(kt, 8)
    c, off = divmod(k8, 4)
    return (c * 2 + r) * 4096 + et * 512 + off * 128


def _build(reps=1):
    nc = bacc.Bacc("TRN2")
    dt_bf16 = mybir.dt.bfloat16
    dt_f32 = mybir.dt.float32

    # partition-major inputs: block dt of X^T/W^T rows [dt*128:(dt+1)*128, :]
    xtq = nc.dram_tensor("xtq", [128, DT * SQ], dt_bf16, kind="ExternalInput")
    xtkv = nc.dram_tensor("xtkv", [128, DT * HALF], dt_bf16, kind="ExternalInput")
    wqt = nc.dram_tensor("wqt", [128, DT * E], dt_bf16, kind="ExternalInput")
    wkt = nc.dram_tensor("wkt", [128, DT * E], dt_bf16, kind="ExternalInput")
    wvt = nc.dram_tensor("wvt", [128, DT * E], dt_bf16, kind="ExternalInput")
    bqs = nc.dram_tensor("bqs", [128, ET], dt_f32, kind="ExternalInput")
    bks = nc.dram_tensor("bks", [128, ET], dt_f32, kind="ExternalInput")
    bvv = nc.dram_tensor("bvv", [1, E], dt_f32, kind="ExternalInput")
    maskt = nc.dram_tensor("maskt", [512, QCH], dt_bf16, kind="ExternalInput")
    o = nc.dram_tensor("o", [SQ, E], dt_bf16, kind="ExternalOutput")

    Ident = mybir.ActivationFunctionType.Identity
    Exp = mybir.ActivationFunctionType.Exp

    with ExitStack() as ctx:
        tc = ctx.enter_context(tile.TileContext(nc))
        persist = ctx.enter_context(tc.tile_pool(name="persist", bufs=1))
        dram = ctx.enter_context(tc.tile_pool(name="dram", bufs=1, space="DRAM"))

        qt = [persist.tile([128, SQ], dt_bf16, tag=f"qt{i}", name=f"qt{i}") for i in range(ET)]
        # K^T, all 16 global key tiles: 4 superblocks [c][r], each [8 et][512]
        ktall = persist.tile([128, 4 * 4096], dt_bf16, tag="ktall")
        # V, all 16 global key tiles: column block kt*1024 is tile kt
        vall = persist.tile([128, KT * E], dt_bf16, tag="vall")
        msk = [persist.tile([128, QCH], dt_bf16, tag=f"m{i}", name=f"m{i}") for i in range(4)]
        bqs_sb = persist.tile([128, ET], dt_f32, tag="bqs")
        bks_sb = persist.tile([128, ET], dt_f32, tag="bks")
        bv_bc = persist.tile([128, E], dt_f32, tag="bvbc")
        ones_col = persist.tile([128, 1], dt_bf16, tag="ones")

        # DRAM bounce buffers for the pairwise K/V exchange (1MB in each)
        kcc_in = [dram.tile([128, 4096], dt_bf16, tag=f"kci{i}", name=f"kci{i}")
                  for i in range(2)]
        kcc_out = [dram.tile([2, 128, 4096], dt_bf16, tag=f"kco{i}", name=f"kco{i}")
                   for i in range(2)]
        vcc_in = [dram.tile([128, 4096], dt_bf16, tag=f"vci{i}", name=f"vci{i}")
                  for i in range(2)]
        vcc_out = [dram.tile([2, 128, 4096], dt_bf16, tag=f"vco{i}", name=f"vco{i}")
                   for i in range(2)]

        nc.vector.memset(ones_col[:], 1.0)

        # repetitions are for timing only (reps>1 cancels dispatch overhead)
        for _rep in range(reps):
            # ---------------- Phase 1: projections + K/V exchange ----------------
            with (
                tc.tile_pool(name="p1", bufs=1) as p1,
                tc.tile_pool(name="psp1", bufs=8, space="PSUM") as psp1,
            ):
                xqa = p1.tile([128, DT * SQ], dt_bf16, tag="xqa")
                xkva = p1.tile([128, DT * HALF], dt_bf16, tag="xkva")
                wqa = p1.tile([128, DT * E], dt_bf16, tag="wqa")
                wka = p1.tile([128, DT * E], dt_bf16, tag="wka")
                wva = p1.tile([128, DT * E], dt_bf16, tag="wva")
                # local-half staging for the exchange, laid out so each CC
                # input copy is one DMA with 8KB-contiguous partition runs
                kstg = [p1.tile([128, 4096], dt_bf16, tag=f"kstg{i}", name=f"kstg{i}")
                        for i in range(2)]
                vstg = [p1.tile([128, 4096], dt_bf16, tag=f"vstg{i}", name=f"vstg{i}")
                        for i in range(2)]

                # warm-up matmuls during the DMA lead-in: ~2.5us of PE
                # activity trips the HAM clock gate to 2.4GHz before the
                # first real matmul arrives
                warm = p1.tile([128, 512], dt_bf16, tag="warm", name="warm")
                nc.vector.memset(warm[:], 0.0)
                wps = psp1.tile([128, 512], dt_f32, tag="ps", name="pswarm")
                for _ in range(12):
                    nc.tensor.matmul(wps[:], warm[:, 0:128], warm[:],
                                     start=True, stop=True)

                # per-dt chunk loads so the PE can stream as data arrives;
                # dt=0 blocks first so the very first matmul starts early
                nc.sync.dma_start(out=xkva[:, 0:HALF], in_=xtkv.ap()[:, 0:HALF])
                nc.sync.dma_start(out=wka[:, 0:E], in_=wkt.ap()[:, 0:E])
                if _rep == 0:
                    nc.sync.dma_start(out=bks_sb[:], in_=bks.ap())
                for i in range(1, DT):
                    nc.sync.dma_start(out=wka[:, i * E:(i + 1) * E],
                                      in_=wkt.ap()[:, i * E:(i + 1) * E])
                    nc.sync.dma_start(out=xkva[:, i * HALF:(i + 1) * HALF],
                                      in_=xtkv.ap()[:, i * HALF:(i + 1) * HALF])
                for i in range(DT):
                    nc.sync.dma_start(out=wva[:, i * E:(i + 1) * E],
                                      in_=wvt.ap()[:, i * E:(i + 1) * E])
                if _rep == 0:
                    bv_ap = bass.AP(tensor=bvv, offset=0, ap=[[0, 128], [1, E]])
                    nc.gpsimd.dma_start(out=bv_bc[:], in_=bv_ap)
                    for i in range(4):
                        nc.sync.dma_start(out=msk[i][:],
                                          in_=maskt.ap()[i * 128:(i + 1) * 128, :])
                    nc.sync.dma_start(out=bqs_sb[:], in_=bqs.ap())

                # K^T local half, column-group outer so each 512-col slab of
                # every e-tile finishes together and is exchanged immediately
                for c in range(2):
                    for et in range(ET):
                        ps = psp1.tile([128, 512], dt_f32, tag="ps")
                        for dt in range(DT):
                            nc.tensor.matmul(
                                ps[:],
                                wka[:, dt * E + et * 128: dt * E + (et + 1) * 128],
                                xkva[:, dt * HALF + c * 512: dt * HALF + (c + 1) * 512],
                                start=(dt == 0), stop=(dt == DT - 1),
                            )
                        nc.scalar.activation(
                            kstg[c][:, et * 512:(et + 1) * 512], ps[:], Ident,
                            bias=bks_sb[:, et:et + 1],
                        )
                    nc.sync.dma_start(out=kcc_in[c][:], in_=kstg[c][:])
                    nc.gpsimd.collective_compute(
                        "AllGather", mybir.AluOpType.bypass,
                        replica_groups=CC_GROUPS,
                        ins=[kcc_in[c][:]], outs=[kcc_out[c][:]],
                    )
                    if c == 1:
                        # Q-projection inputs: needed only when Q starts
                        # (~65us); emitting them behind the second CC input
                        # copy keeps the rings clear for the K-half load and
                        # the exchange traffic
                        for i in range(DT):
                            nc.sync.dma_start(out=wqa[:, i * E:(i + 1) * E],
                                              in_=wqt.ap()[:, i * E:(i + 1) * E])
                            nc.sync.dma_start(out=xqa[:, i * SQ:(i + 1) * SQ],
                                              in_=xtq.ap()[:, i * SQ:(i + 1) * SQ])

                # V local half (tile j of this core's half; global tile is
                # rank*8+j). vstg[g] holds 4 tiles as 1024-col blocks.
                for g in range(2):
                    for j in range(g * 4, g * 4 + 4):
                        for c2 in range(2):
                            ps = psp1.tile([128, 512], dt_f32, tag="ps")
                            for dt in range(DT):
                                nc.tensor.matmul(
                                    ps[:],
                                    xkva[:, dt * HALF + j * 128: dt * HALF + (j + 1) * 128],
                                    wva[:, dt * E + c2 * 512: dt * E + (c2 + 1) * 512],
                                    start=(dt == 0), stop=(dt == DT - 1),
                                )
                            nc.vector.tensor_add(
                                vstg[g][:, (j - g * 4) * E + c2 * 512:
                                          (j - g * 4) * E + (c2 + 1) * 512],
                                ps[:],
                                bv_bc[:, c2 * 512:(c2 + 1) * 512],
                            )
                    nc.sync.dma_start(out=vcc_in[g][:], in_=vstg[g][:])
                    nc.gpsimd.collective_compute(
                        "AllGather", mybir.AluOpType.bypass,
                        replica_groups=CC_GROUPS,
                        ins=[vcc_in[g][:]], outs=[vcc_out[g][:]],
                    )

                # Q^T[e, sq] = (Wq/sqrt(D))^T.T @ X^T  (+ bq/sqrt(D))
                for cq in range(2):
                    for et in range(ET):
                        ps = psp1.tile([128, 512], dt_f32, tag="ps")
                        for dt in range(DT):
                            nc.tensor.matmul(
                                ps[:],
                                wqa[:, dt * E + et * 128: dt * E + (et + 1) * 128],
                                xqa[:, dt * SQ + cq * 512: dt * SQ + (cq + 1) * 512],
                                start=(dt == 0), stop=(dt == DT - 1),
                            )
                        nc.scalar.activation(
                            qt[et][:, cq * 512:(cq + 1) * 512], ps[:], Ident,
                            bias=bqs_sb[:, et:et + 1],
                        )

                # CC readbacks, all at the end of phase 1: each waits on its
                # collective, but everything independent has already been
                # dispatched ahead of them on the in-order sync queue.
                # One DMA per (slab, rank): 8KB contiguous per partition.
                for c in range(2):
                    for r in range(2):
                        nc.sync.dma_start(
                            out=ktall[:, (c * 2 + r) * 4096:(c * 2 + r + 1) * 4096],
                            in_=kcc_out[c][r, :, :],
                        )
                for g in range(2):
                    for r in range(2):
                        base = (r * 8 + g * 4) * E
                        nc.sync.dma_start(
                            out=vall[:, base:base + 4096],
                            in_=vcc_out[g][r, :, :],
                        )

            # ---------------- Phase 2: attention ----------------
            with (
                tc.tile_pool(name="p2", bufs=1) as p2,
                tc.tile_pool(name="pss", bufs=2, space="PSUM") as pss,
                tc.tile_pool(name="psd", bufs=2, space="PSUM") as psd,
                tc.tile_pool(name="pso", bufs=2, space="PSUM") as pso,
            ):
                for c in range(NCH):
                    nkt = 4 * (c + 1)
                    qc = c * QCH
                    # scores S^T[k, q] for this chunk, then p = exp
                    pts = []
                    for kt in range(nkt):
                        sps = pss.tile([128, QCH], dt_f32, tag="st")
                        for et in range(ET):
                            kc = _kcol(et, kt)
                            nc.tensor.matmul(
                                sps[:],
                                ktall[:, kc:kc + 128],
                                qt[et][:, qc:qc + QCH],
                                start=(et == 0), stop=(et == ET - 1),
                            )
                        pt = p2.tile([128, QCH], dt_bf16, tag="pt", bufs=32)
                        nc.scalar.activation(pt[:], sps[:], Exp)
                        if kt >= nkt - 4:
                            nc.vector.tensor_mul(pt[:], pt[:], msk[kt - (nkt - 4)][:])
                        pts.append(pt)
                    # PV + denominator
                    for h in range(2):
                        ops = pso.tile([128, E], dt_f32, tag="o")
                        dps = psd.tile([128, 8], dt_f32, tag="d")
                        hs = slice(h * 128, (h + 1) * 128)
                        for kt in range(nkt):
                            st = (kt == 0)
                            sp = (kt == nkt - 1)
                            vb = kt * E
                            nc.tensor.matmul(ops[:, 0:512], pts[kt][:, hs],
                                             vall[:, vb:vb + 512], start=st, stop=sp)
                            nc.tensor.matmul(ops[:, 512:1024], pts[kt][:, hs],
                                             vall[:, vb + 512:vb + 1024], start=st, stop=sp)
                            nc.tensor.matmul(dps[:, 0:1], pts[kt][:, hs],
                                             ones_col[:], start=st, stop=sp)
                        den_r = p2.tile([128, 1], dt_f32, tag="denr", bufs=2)
                        nc.vector.reciprocal(den_r[:], dps[:, 0:1])
                        o_sb = p2.tile([128, E], dt_bf16, tag="osb", bufs=4)
                        # store in column halves so the DMA of the first half
                        # overlaps the divide of the second (tail latency)
                        for oh in range(2):
                            os_ = slice(oh * 512, (oh + 1) * 512)
                            nc.vector.tensor_scalar_mul(o_sb[:, os_], ops[:, os_], den_r[:])
                            nc.sync.dma_start(
                                out=o.ap()[qc + h * 128: qc + (h + 1) * 128, os_],
                                in_=o_sb[:, os_],
                            )

    nc.compile()
    return nc


def _pm(a):
    """[DT*128, N] -> partition-major [128, DT*N] (block dt = rows dt*128..)."""
    n = a.shape[1]
    return np.ascontiguousarray(
        a.reshape(DT, 128, n).transpose(1, 0, 2).reshape(128, DT * n)
    )


def _host_shard(inputs, Wq, bq, Wk, bk, Wv, bv):
    """Build the 8 per-core input maps."""
    scale = np.sqrt(np.float32(D))
    wqt = _pm(np.ascontiguousarray((Wq / scale).T).astype(bf16))
    wkt = _pm(np.ascontiguousarray(Wk.T).astype(bf16))
    wvt = _pm(np.ascontiguousarray(Wv.T).astype(bf16))
    bqs = np.ascontiguousarray((bq / scale).reshape(ET, 128).T).astype(f32)
    bks = np.ascontiguousarray(bk.reshape(ET, 128).T).astype(f32)
    bvv = np.ascontiguousarray(bv.reshape(1, E)).astype(f32)

    # masks: [512 keys, 256 q], multiplicative
    kk = np.arange(512)[:, None]
    qq = np.arange(QCH)[None, :]
    mask_p0 = np.where(kk < 256, (kk <= qq), False).astype(bf16)
    mask_p1 = np.where(kk < 256, True, (kk - 256) <= qq).astype(bf16)
    masks = [mask_p0, mask_p1]

    in_maps = []
    for core in range(N_CORES):
        b, p = divmod(core, 2)
        xb = inputs[b]                       # [S, D] fp32
        rows = np.concatenate(
            [xb[QCH * (2 * c + p): QCH * (2 * c + p) + QCH] for c in range(NCH)],
            axis=0,
        )                                    # [SQ, D]
        in_maps.append({
            "xtq": _pm(np.ascontiguousarray(rows.T).astype(bf16)),
            "xtkv": _pm(np.ascontiguousarray(
                xb[HALF * p: HALF * (p + 1)].T).astype(bf16)),
            "wqt": wqt, "wkt": wkt, "wvt": wvt,
            "bqs": bqs, "bks": bks, "bvv": bvv,
            "maskt": masks[p],
        })
    return in_maps


def _assemble(results, dtype):
    out = np.empty((B, S, E), dtype=dtype)
    for core in range(N_CORES):
        b, p = divmod(core, 2)
        oc = results[core]["o"].astype(dtype)
        for c in range(NCH):
            g = 2 * c + p
            out[b, QCH * g: QCH * (g + 1)] = oc[QCH * c: QCH * (c + 1)]
    return out


def kernel(inputs, Wq, bq, Wk, bk, Wv, bv):
    inputs = np.asarray(inputs, dtype=f32)
    Wq, bq = np.asarray(Wq, dtype=f32), np.asarray(bq, dtype=f32)
    Wk, bk = np.asarray(Wk, dtype=f32), np.asarray(bk, dtype=f32)
    Wv, bv = np.asarray(Wv, dtype=f32), np.asarray(bv, dtype=f32)

    if "nc" not in _CACHE:
        _CACHE["nc"] = _build()
    nc = _CACHE["nc"]

    in_maps = _host_shard(inputs, Wq, bq, Wk, bk, Wv, bv)
    res = bass_utils.run_bass_kernel_spmd(nc, in_maps, core_ids=list(range(N_CORES)))
    return _assemble(res.results, f32)


# revision 4
# speedup vs baseline: 1.1920x; 1.1920x over previous
"""Causal single-head attention (B=4, S=2048, D=1024) on 8 trn2 NeuronCores.

Sharding: 8 cores = 4 batches x 2 sequence-shards. Queries are split into
sixteen 128-row blocks per batch; core (b, p) handles blocks of parity p
(interleaving balances causal work). Attention processes its 8 blocks as 4
pairs: the pair's common causal key range runs at full 256-wide moving
efficiency and only the second block's 2 extra diagonal key tiles run
128-wide — exact causal tiling, 36 key-tile units per core instead of 40.
K/V projections are NOT duplicated: core (b, p) projects K/V only for keys
[1024p, 1024p+1024), then the pair exchanges halves with four pipelined 1MB
pairwise AllGathers (DRAM bounce) that complete while the PE continues with
V/Q projections.

DMA discipline (the rings, not bytes, were the bottleneck):
  - every 2MB input is pre-laid-out on the host partition-major so it loads
    with ONE dma (16KB contiguous per partition run)
  - every CC input copy / readback is one DMA with 8KB runs (staging tiles
    kstg/vstg; wide ktall/vall layouts)
  - all CC readbacks sit at the END of phase 1 so the in-order sync queue
    never blocks independent work behind a collective wait
  - wq/xtq loads are emitted behind the first CC input copy: they dispatch
    ~25us in, keeping rings clear for the K-half load + exchange
"""

import numpy as np
import ml_dtypes
from contextlib import ExitStack

import concourse.bacc as bacc
import concourse.bass as bass
import concourse.mybir as mybir
import concourse.tile as tile
from concourse import bass_utils

bf16 = ml_dtypes.bfloat16
f32 = np.float32

B, S, D = 4, 2048, 1024
E = D
N_CORES = 8
QCH = 256          # query chunk rows (per-core local chunk)
NCH = 4            # local chunks per core
SQ = QCH * NCH     # 1024 query rows per core
HALF = S // 2      # keys projected locally per core
DT = D // 128      # 8 d-tiles
ET = E // 128      # 8 e-tiles
KT = S // 128      # 16 key tiles (global)
CC_GROUPS = [[0, 1], [2, 3], [4, 5], [6, 7]]

_CACHE = {}


def _kcol(et, kt):
    """Column of global K tile (et, kt) in the ktall layout [c][r][et][512]."""
    r, k8 = divmod(kt, 8)
    c, off = divmod(k8, 4)
    return (c * 2 + r) * 4096 + et * 512 + off * 128


def _build(reps=1):
    nc = bacc.Bacc("TRN2")
    dt_bf16 = mybir.dt.bfloat16
    dt_f32 = mybir.dt.float32

    # partition-major inputs: block dt of X^T/W^T rows [dt*128:(dt+1)*128, :]
    xtq = nc.dram_tensor("xtq", [128, DT * SQ], dt_bf16, kind="ExternalInput")
    xtkv = nc.dram_tensor("xtkv", [128, DT * HALF], dt_bf16, kind="ExternalInput")
    wqt = nc.dram_tensor("wqt", [128, DT * E], dt_bf16, kind="ExternalInput")
    wkt = nc.dram_tensor("wkt", [128, DT * E], dt_bf16, kind="ExternalInput")
    wvt = nc.dram_tensor("wvt", [128, DT * E], dt_bf16, kind="ExternalInput")
    bqs = nc.dram_tensor("bqs", [128, ET], dt_f32, kind="ExternalInput")
    bks = nc.dram_tensor("bks", [128, ET], dt_f32, kind="ExternalInput")
    bvv = nc.dram_tensor("bvv", [1, E], dt_f32, kind="ExternalInput")
    maskt = nc.dram_tensor("maskt", [256, 128], dt_bf16, kind="ExternalInput")
    o = nc.dram_tensor("o", [SQ, E], dt_bf16, kind="ExternalOutput")

    Ident = mybir.ActivationFunctionType.Identity
    Exp = mybir.ActivationFunctionType.Exp

    with ExitStack() as ctx:
        tc = ctx.enter_context(tile.TileContext(nc))
        persist = ctx.enter_context(tc.tile_pool(name="persist", bufs=1))
        dram = ctx.enter_context(tc.tile_pool(name="dram", bufs=1, space="DRAM"))

        qt = [persist.tile([128, SQ], dt_bf16, tag=f"qt{i}", name=f"qt{i}") for i in range(ET)]
        # K^T, all 16 global key tiles: 4 superblocks [c][r], each [8 et][512]
        ktall = persist.tile([128, 4 * 4096], dt_bf16, tag="ktall")
        # V, all 16 global key tiles: column block kt*1024 is tile kt
        vall = persist.tile([128, KT * E], dt_bf16, tag="vall")
        msk = [persist.tile([128, 128], dt_bf16, tag=f"m{i}", name=f"m{i}") for i in range(2)]
        bqs_sb = persist.tile([128, ET], dt_f32, tag="bqs")
        bks_sb = persist.tile([128, ET], dt_f32, tag="bks")
        bv_bc = persist.tile([128, E], dt_f32, tag="bvbc")
        ones_col = persist.tile([128, 1], dt_bf16, tag="ones")

        # DRAM bounce buffers for the pairwise K/V exchange (1MB in each)
        kcc_in = [dram.tile([128, 4096], dt_bf16, tag=f"kci{i}", name=f"kci{i}")
                  for i in range(2)]
        kcc_out = [dram.tile([2, 128, 4096], dt_bf16, tag=f"kco{i}", name=f"kco{i}")
                   for i in range(2)]
        vcc_in = [dram.tile([128, 4096], dt_bf16, tag=f"vci{i}", name=f"vci{i}")
                  for i in range(2)]
        vcc_out = [dram.tile([2, 128, 4096], dt_bf16, tag=f"vco{i}", name=f"vco{i}")
                   for i in range(2)]

        nc.vector.memset(ones_col[:], 1.0)

        # repetitions are for timing only (reps>1 cancels dispatch overhead)
        for _rep in range(reps):
            # ---------------- Phase 1: projections + K/V exchange ----------------
            with (
                tc.tile_pool(name="p1", bufs=1) as p1,
                tc.tile_pool(name="psp1", bufs=8, space="PSUM") as psp1,
            ):
                xqa = p1.tile([128, DT * SQ], dt_bf16, tag="xqa")
                xkva = p1.tile([128, DT * HALF], dt_bf16, tag="xkva")
                wqa = p1.tile([128, DT * E], dt_bf16, tag="wqa")
                wka = p1.tile([128, DT * E], dt_bf16, tag="wka")
                wva = p1.tile([128, DT * E], dt_bf16, tag="wva")
                # local-half staging for the exchange, laid out so each CC
                # input copy is one DMA with 8KB-contiguous partition runs
                kstg = [p1.tile([128, 4096], dt_bf16, tag=f"kstg{i}", name=f"kstg{i}")
                        for i in range(2)]
                vstg = [p1.tile([128, 4096], dt_bf16, tag=f"vstg{i}", name=f"vstg{i}")
                        for i in range(2)]

                # warm-up matmuls during the DMA lead-in: ~2.5us of PE
                # activity trips the HAM clock gate to 2.4GHz before the
                # first real matmul arrives
                warm = p1.tile([128, 512], dt_bf16, tag="warm", name="warm")
                nc.vector.memset(warm[:], 0.0)
                wps = psp1.tile([128, 512], dt_f32, tag="ps", name="pswarm")
                for _ in range(12):
                    nc.tensor.matmul(wps[:], warm[:, 0:128], warm[:],
                                     start=True, stop=True)

                # per-dt chunk loads so the PE can stream as data arrives;
                # dt=0 blocks first so the very first matmul starts early
                nc.sync.dma_start(out=xkva[:, 0:HALF], in_=xtkv.ap()[:, 0:HALF])
                nc.sync.dma_start(out=wka[:, 0:E], in_=wkt.ap()[:, 0:E])
                if _rep == 0:
                    nc.sync.dma_start(out=bks_sb[:], in_=bks.ap())
                for i in range(1, DT):
                    nc.sync.dma_start(out=wka[:, i * E:(i + 1) * E],
                                      in_=wkt.ap()[:, i * E:(i + 1) * E])
                    nc.sync.dma_start(out=xkva[:, i * HALF:(i + 1) * HALF],
                                      in_=xtkv.ap()[:, i * HALF:(i + 1) * HALF])
                for i in range(DT):
                    nc.sync.dma_start(out=wva[:, i * E:(i + 1) * E],
                                      in_=wvt.ap()[:, i * E:(i + 1) * E])
                if _rep == 0:
                    bv_ap = bass.AP(tensor=bvv, offset=0, ap=[[0, 128], [1, E]])
                    nc.gpsimd.dma_start(out=bv_bc[:], in_=bv_ap)
                    for i in range(2):
                        nc.sync.dma_start(out=msk[i][:],
                                          in_=maskt.ap()[i * 128:(i + 1) * 128, :])
                    nc.sync.dma_start(out=bqs_sb[:], in_=bqs.ap())

                # K^T local half, column-group outer so each 512-col slab of
                # every e-tile finishes together and is exchanged immediately
                for c in range(2):
                    for et in range(ET):
                        ps = psp1.tile([128, 512], dt_f32, tag="ps")
                        for dt in range(DT):
                            nc.tensor.matmul(
                                ps[:],
                                wka[:, dt * E + et * 128: dt * E + (et + 1) * 128],
                                xkva[:, dt * HALF + c * 512: dt * HALF + (c + 1) * 512],
                                start=(dt == 0), stop=(dt == DT - 1),
                            )
                        nc.scalar.activation(
                            kstg[c][:, et * 512:(et + 1) * 512], ps[:], Ident,
                            bias=bks_sb[:, et:et + 1],
                        )
                    nc.sync.dma_start(out=kcc_in[c][:], in_=kstg[c][:])
                    nc.gpsimd.collective_compute(
                        "AllGather", mybir.AluOpType.bypass,
                        replica_groups=CC_GROUPS,
                        ins=[kcc_in[c][:]], outs=[kcc_out[c][:]],
                    )
                    if c == 1:
                        # Q-projection inputs: needed only when Q starts
                        # (~65us); emitting them behind the second CC input
                        # copy keeps the rings clear for the K-half load and
                        # the exchange traffic
                        for i in range(DT):
                            nc.sync.dma_start(out=wqa[:, i * E:(i + 1) * E],
                                              in_=wqt.ap()[:, i * E:(i + 1) * E])
                            nc.sync.dma_start(out=xqa[:, i * SQ:(i + 1) * SQ],
                                              in_=xtq.ap()[:, i * SQ:(i + 1) * SQ])

                # V local half (tile j of this core's half; global tile is
                # rank*8+j). vstg[g] holds 4 tiles as 1024-col blocks.
                for g in range(2):
                    for j in range(g * 4, g * 4 + 4):
                        for c2 in range(2):
                            ps = psp1.tile([128, 512], dt_f32, tag="ps")
                            for dt in range(DT):
                                nc.tensor.matmul(
                                    ps[:],
                                    xkva[:, dt * HALF + j * 128: dt * HALF + (j + 1) * 128],
                                    wva[:, dt * E + c2 * 512: dt * E + (c2 + 1) * 512],
                                    start=(dt == 0), stop=(dt == DT - 1),
                                )
                            nc.vector.tensor_add(
                                vstg[g][:, (j - g * 4) * E + c2 * 512:
                                          (j - g * 4) * E + (c2 + 1) * 512],
                                ps[:],
                                bv_bc[:, c2 * 512:(c2 + 1) * 512],
                            )
                    nc.sync.dma_start(out=vcc_in[g][:], in_=vstg[g][:])
                    nc.gpsimd.collective_compute(
                        "AllGather", mybir.AluOpType.bypass,
                        replica_groups=CC_GROUPS,
                        ins=[vcc_in[g][:]], outs=[vcc_out[g][:]],
                    )

                # Q^T[e, sq] = (Wq/sqrt(D))^T.T @ X^T  (+ bq/sqrt(D))
                for cq in range(2):
                    for et in range(ET):
                        ps = psp1.tile([128, 512], dt_f32, tag="ps")
                        for dt in range(DT):
                            nc.tensor.matmul(
                                ps[:],
                                wqa[:, dt * E + et * 128: dt * E + (et + 1) * 128],
                                xqa[:, dt * SQ + cq * 512: dt * SQ + (cq + 1) * 512],
                                start=(dt == 0), stop=(dt == DT - 1),
                            )
                        nc.scalar.activation(
                            qt[et][:, cq * 512:(cq + 1) * 512], ps[:], Ident,
                            bias=bqs_sb[:, et:et + 1],
                        )

                # CC readbacks, all at the end of phase 1: each waits on its
                # collective, but everything independent has already been
                # dispatched ahead of them on the in-order sync queue.
                # One DMA per (slab, rank): 8KB contiguous per partition.
                for c in range(2):
                    for r in range(2):
                        nc.sync.dma_start(
                            out=ktall[:, (c * 2 + r) * 4096:(c * 2 + r + 1) * 4096],
                            in_=kcc_out[c][r, :, :],
                        )
                for g in range(2):
                    for r in range(2):
                        base = (r * 8 + g * 4) * E
                        nc.sync.dma_start(
                            out=vall[:, base:base + 4096],
                            in_=vcc_out[g][r, :, :],
                        )

            # ---------------- Phase 2: attention ----------------
            with (
                tc.tile_pool(name="p2", bufs=1) as p2,
                tc.tile_pool(name="pss", bufs=2, space="PSUM") as pss,
                tc.tile_pool(name="psd", bufs=2, space="PSUM") as psd,
                tc.tile_pool(name="pso", bufs=2, space="PSUM") as pso,
            ):
                for pi in range(4):
                    i = 2 * pi          # first local 128-row chunk of the pair
                    ncommon = 2 * i + 2
                    qc = i * 128
                    # scores for the pair's common key range at 256-wide
                    ptsc = []
                    for kt in range(ncommon):
                        sps = pss.tile([128, 256], dt_f32, tag="st")
                        for et in range(ET):
                            kc = _kcol(et, kt)
                            nc.tensor.matmul(
                                sps[:],
                                ktall[:, kc:kc + 128],
                                qt[et][:, qc:qc + 256],
                                start=(et == 0), stop=(et == ET - 1),
                            )
                        pt = p2.tile([128, 256], dt_bf16, tag="pt", bufs=32)
                        nc.scalar.activation(pt[:], sps[:], Exp)
                        ptsc.append(pt)
                    # second chunk's two extra diagonal tiles at 128-wide
                    ptse = []
                    for kt in range(ncommon, ncommon + 2):
                        sps = pss.tile([128, 256], dt_f32, tag="st")
                        for et in range(ET):
                            kc = _kcol(et, kt)
                            nc.tensor.matmul(
                                sps[:, 0:128],
                                ktall[:, kc:kc + 128],
                                qt[et][:, qc + 128:qc + 256],
                                start=(et == 0), stop=(et == ET - 1),
                            )
                        pt = p2.tile([128, 128], dt_bf16, tag="pte", bufs=8)
                        nc.scalar.activation(pt[:], sps[:, 0:128], Exp)
                        ptse.append(pt)
                    # causal masks: each chunk's last two emitted tiles
                    nc.vector.tensor_mul(ptsc[ncommon - 2][:, 0:128],
                                         ptsc[ncommon - 2][:, 0:128], msk[0][:])
                    nc.vector.tensor_mul(ptsc[ncommon - 1][:, 0:128],
                                         ptsc[ncommon - 1][:, 0:128], msk[1][:])
                    nc.vector.tensor_mul(ptse[0][:], ptse[0][:], msk[0][:])
                    nc.vector.tensor_mul(ptse[1][:], ptse[1][:], msk[1][:])
                    # PV + denominator per chunk (half 0 = chunk i, 1 = i+1)
                    for half in range(2):
                        nwk = ncommon + 2 * half
                        ops = pso.tile([128, E], dt_f32, tag="o")
                        dps = psd.tile([128, 8], dt_f32, tag="d")
                        hs = slice(half * 128, half * 128 + 128)
                        for kt in range(nwk):
                            st = (kt == 0)
                            sp = (kt == nwk - 1)
                            stat = (ptsc[kt][:, hs] if kt < ncommon
                                    else ptse[kt - ncommon][:])
                            vb = kt * E
                            nc.tensor.matmul(ops[:, 0:512], stat,
                                             vall[:, vb:vb + 512], start=st, stop=sp)
                            nc.tensor.matmul(ops[:, 512:1024], stat,
                                             vall[:, vb + 512:vb + 1024], start=st, stop=sp)
                            nc.tensor.matmul(dps[:, 0:1], stat,
                                             ones_col[:], start=st, stop=sp)
                        den_r = p2.tile([128, 1], dt_f32, tag="denr", bufs=8)
                        nc.vector.reciprocal(den_r[:], dps[:, 0:1])
                        o_sb = p2.tile([128, E], dt_bf16, tag="osb", bufs=4)
                        # store in column halves so the DMA of the first half
                        # overlaps the divide of the second (tail latency)
                        for oh in range(2):
                            os_ = slice(oh * 512, (oh + 1) * 512)
                            nc.vector.tensor_scalar_mul(o_sb[:, os_], ops[:, os_], den_r[:])
                            nc.sync.dma_start(
                                out=o.ap()[qc + half * 128: qc + (half + 1) * 128, os_],
                                in_=o_sb[:, os_],
                            )

    nc.compile()
    return nc


def _pm(a):
    """[DT*128, N] -> partition-major [128, DT*N] (block dt = rows dt*128..)."""
    n = a.shape[1]
    return np.ascontiguousarray(
        a.reshape(DT, 128, n).transpose(1, 0, 2).reshape(128, DT * n)
    )


def _host_shard(inputs, Wq, bq, Wk, bk, Wv, bv):
    """Build the 8 per-core input maps."""
    scale = np.sqrt(np.float32(D))
    wqt = _pm(np.ascontiguousarray((Wq / scale).T).astype(bf16))
    wkt = _pm(np.ascontiguousarray(Wk.T).astype(bf16))
    wvt = _pm(np.ascontiguousarray(Wv.T).astype(bf16))
    bqs = np.ascontiguousarray((bq / scale).reshape(ET, 128).T).astype(f32)
    bks = np.ascontiguousarray(bk.reshape(ET, 128).T).astype(f32)
    bvv = np.ascontiguousarray(bv.reshape(1, E)).astype(f32)

    # masks [256 keys, 128 q]: rows 0-127 apply to each chunk's
    # second-to-last emitted key tile, rows 128-255 to the last.
    kk = np.arange(128)[:, None]
    qq = np.arange(128)[None, :]
    tri = (kk <= qq).astype(bf16)
    mask_p0 = np.concatenate([tri, np.zeros((128, 128), dtype=bf16)])
    mask_p1 = np.concatenate([np.ones((128, 128), dtype=bf16), tri])
    masks = [mask_p0, mask_p1]

    in_maps = []
    for core in range(N_CORES):
        b, p = divmod(core, 2)
        xb = inputs[b]                       # [S, D] fp32
        rows = np.concatenate(
            [xb[128 * (2 * c + p): 128 * (2 * c + p) + 128] for c in range(8)],
            axis=0,
        )                                    # [SQ, D]
        in_maps.append({
            "xtq": _pm(np.ascontiguousarray(rows.T).astype(bf16)),
            "xtkv": _pm(np.ascontiguousarray(
                xb[HALF * p: HALF * (p + 1)].T).astype(bf16)),
            "wqt": wqt, "wkt": wkt, "wvt": wvt,
            "bqs": bqs, "bks": bks, "bvv": bvv,
            "maskt": masks[p],
        })
    return in_maps


def _assemble(results, dtype):
    out = np.empty((B, S, E), dtype=dtype)
    for core in range(N_CORES):
        b, p = divmod(core, 2)
        oc = results[core]["o"].astype(dtype)
        for c in range(8):
            g = 2 * c + p
            out[b, 128 * g: 128 * (g + 1)] = oc[128 * c: 128 * (c + 1)]
    return out


def kernel(inputs, Wq, bq, Wk, bk, Wv, bv):
    inputs = np.asarray(inputs, dtype=f32)
    Wq, bq = np.asarray(Wq, dtype=f32), np.asarray(bq, dtype=f32)
    Wk, bk = np.asarray(Wk, dtype=f32), np.asarray(bk, dtype=f32)
    Wv, bv = np.asarray(Wv, dtype=f32), np.asarray(bv, dtype=f32)

    if "nc" not in _CACHE:
        _CACHE["nc"] = _build()
    nc = _CACHE["nc"]

    in_maps = _host_shard(inputs, Wq, bq, Wk, bk, Wv, bv)
    res = bass_utils.run_bass_kernel_spmd(nc, in_maps, core_ids=list(range(N_CORES)))
    return _assemble(res.results, f32)
